# revision 8
# baseline (speedup 1.0000x reference)
"""Trainium2 Bass kernel for BaseAttentionBlock (B=8, C=512, HxW=64x64, K=V=256, O=512).

Strategy: data-parallel over batch B across the 8 NeuronCores (one batch element
per core, SPMD, no collectives). Per core:

  k' = relu(s*(wk@x) + b2)/4 (BN folded on host, 1/sqrt(K) folded as 1/4 into k')
  vT = (x^T @ wv^T) + bv     computed directly in [m, v] layout (no transposes)
  E  = exp(k'^T k')          symmetric, so the [128m x 512n] tile computed in the
                             fused loop is simultaneously the [m, n]-layout rhs the
                             ctx matmul needs -> single pass, no transposes, no
                             DRAM round trip, rowsums fused into exp via accum_out
  ctx_raw[v, n] = sum_m vT[m, v] * E[m, n]   accumulated in PSUM per 512-wide n chunk
  out[o, n] = wW @ (ctx_raw * (1/rowsum)) + bW

All heavy matmuls in bf16 (PE 1 cycle/row), accumulation fp32 in PSUM.
"""

import numpy as np
import ml_dtypes

BN_EPS = 1e-5
B, C, H, W = 8, 512, 64, 64
N = H * W  # 4096
K = 256
V = 256
O = 512
P = 128  # partitions
NT = 512  # free-dim tile
CC = C // P  # 4 contraction chunks for projections
KC = K // P  # 2 kch chunks
MI = N // P  # 32 m-chunks of 128
NJ = N // NT  # 8 n-chunks of 512
BF16 = ml_dtypes.bfloat16

_COMPILED = None  # (nc, input_names) cache


def _build():
    import concourse.bass as bass
    import concourse.tile as tile
    import concourse.mybir as mybir
    from concourse import bacc, masks
    from contextlib import ExitStack

    f32 = mybir.dt.float32
    bf16 = mybir.dt.bfloat16
    AF = mybir.ActivationFunctionType

    nc = bacc.Bacc(trn_type="TRN2", target_bir_lowering=False, debug=False,
                   num_devices=B)

    x_d = nc.dram_tensor("x16", [C, N], bf16, kind="ExternalInput").ap()
    wkT_d = nc.dram_tensor("wkT16", [C, K], bf16, kind="ExternalInput").ap()
    wvT_d = nc.dram_tensor("wvT16", [C, V], bf16, kind="ExternalInput").ap()
    wWT_d = nc.dram_tensor("wWT16", [V, O], bf16, kind="ExternalInput").ap()
    ks_d = nc.dram_tensor("kscale", [K, 1], f32, kind="ExternalInput").ap()
    kb_d = nc.dram_tensor("kbias", [K, 1], f32, kind="ExternalInput").ap()
    bv_d = nc.dram_tensor("bvrow", [1, V], f32, kind="ExternalInput").ap()
    bW_d = nc.dram_tensor("bW32", [O, 1], f32, kind="ExternalInput").ap()
    out_d = nc.dram_tensor("out", [O, N], f32, kind="ExternalOutput").ap()

    with tile.TileContext(nc) as tc, ExitStack() as ctx:
        const = ctx.enter_context(tc.tile_pool(name="const", bufs=1))

        # ---- persistent SBUF tensors ----
        x_sb = [const.tile([P, N], bf16, tag=f"x{c}", name=f"x_sb{c}")
                for c in range(CC)]
        wk_sb = [const.tile([P, K], bf16, tag=f"wk{c}", name=f"wk_sb{c}")
                 for c in range(CC)]
        wv_sb = [const.tile([P, V], bf16, tag=f"wv{c}", name=f"wv_sb{c}")
                 for c in range(CC)]
        wW_sb = [const.tile([P, O], bf16, tag=f"wW{v}", name=f"wW_sb{v}")
                 for v in range(KC)]
        ks_sb = const.tile([P, KC], f32, tag="ks", name="ks_sb")
        kb_sb = const.tile([P, KC], f32, tag="kb", name="kb_sb")
        bvrow_sb = const.tile([1, V], f32, tag="bvrow", name="bvrow_sb")
        bW_sb = const.tile([P, O // P], f32, tag="bW", name="bW_sb")
        ones32 = const.tile([1, P], f32, tag="ones32", name="ones32")
        ident = const.tile([P, P], f32, tag="ident", name="ident")

        k_sb = [const.tile([P, N], bf16, tag=f"k{kc}", name=f"k_sb{kc}")
                for kc in range(KC)]
        vT_sb = const.tile([P, MI * V], bf16, tag="vT", name="vT_sb")
        ctxr_sb = [const.tile([P, N], f32, tag=f"ctxr{v}", name=f"ctxr_sb{v}")
                   for v in range(KC)]
        rsparts_sb = const.tile([P, MI * NJ], f32, tag="rsparts",
                                name="rsparts_sb")
        rs_sb = const.tile([P, MI], f32, tag="rs", name="rs_sb")
        recipT_sb = const.tile([MI, P], f32, tag="recipT", name="recipT_sb")
        reciprow_sb = const.tile([1, N], f32, tag="reciprow", name="reciprow_sb")
        bvbc_sb = const.tile([P, V], f32, tag="bvbc", name="bvbc_sb")

        # ---- input DMAs ----
        for c in range(CC):
            nc.sync.dma_start(out=x_sb[c][:], in_=x_d[c * P:(c + 1) * P, :])
            nc.sync.dma_start(out=wk_sb[c][:], in_=wkT_d[c * P:(c + 1) * P, :])
            nc.sync.dma_start(out=wv_sb[c][:], in_=wvT_d[c * P:(c + 1) * P, :])
        for v in range(KC):
            nc.sync.dma_start(out=wW_sb[v][:], in_=wWT_d[v * P:(v + 1) * P, :])
        for kc in range(KC):
            nc.sync.dma_start(out=ks_sb[:, kc:kc + 1],
                              in_=ks_d[kc * P:(kc + 1) * P, :])
            nc.sync.dma_start(out=kb_sb[:, kc:kc + 1],
                              in_=kb_d[kc * P:(kc + 1) * P, :])
        nc.sync.dma_start(out=bvrow_sb[:], in_=bv_d[:])
        for oc in range(O // P):
            nc.sync.dma_start(out=bW_sb[:, oc:oc + 1],
                              in_=bW_d[oc * P:(oc + 1) * P, :])
        nc.gpsimd.memset(ones32[:], 1.0)
        masks.make_identity(nc, ident[:])
        # bv broadcast [P, V] via stride-0 DMA read from DRAM
        nc.sync.dma_start(out=bvbc_sb[:], in_=bv_d[:].to_broadcast((P, V)))

        # ---- phase A: projections ----
        with tc.tile_pool(name="psA", bufs=4, space="PSUM") as psA:

            # k' = relu(ks * (wk@x) + kb), bf16, [K, N] as 2 partition chunks
            for kc in range(KC):
                for nj in range(NJ):
                    kp_ps = psA.tile([P, NT], f32, tag="ps", name="kp_ps")
                    for c in range(CC):
                        nc.tensor.matmul(
                            kp_ps[:],
                            lhsT=wk_sb[c][:, kc * P:(kc + 1) * P],
                            rhs=x_sb[c][:, nj * NT:(nj + 1) * NT],
                            start=(c == 0), stop=(c == CC - 1))
                    nc.scalar.activation(
                        k_sb[kc][:, nj * NT:(nj + 1) * NT], kp_ps[:],
                        AF.Relu, bias=kb_sb[:, kc:kc + 1],
                        scale=ks_sb[:, kc:kc + 1])

            # vT = x^T @ wvT + bv, bf16, [m, v] layout: 32 chunks of [128, 256]
            for mi in range(MI):
                vp_ps = psA.tile([P, NT], f32, tag="ps", name="vp_ps")
                for c in range(CC):
                    nc.tensor.matmul(
                        vp_ps[:, :V],
                        lhsT=x_sb[c][:, mi * P:(mi + 1) * P],
                        rhs=wv_sb[c][:],
                        start=(c == 0), stop=(c == CC - 1))
                nc.vector.tensor_add(vT_sb[:, mi * V:(mi + 1) * V],
                                     vp_ps[:, :V], bvbc_sb[:])

        # ---- main fused loop: sim -> exp(+rowsum) -> ctx accumulate ----
        with tc.tile_pool(name="psS", bufs=3, space="PSUM") as psS, \
                tc.tile_pool(name="psC", bufs=4, space="PSUM") as psC, \
                tc.tile_pool(name="epool", bufs=6) as epool:
            for nj in range(NJ):
                ctx_ps = [psC.tile([P, NT], f32, tag="ctx", name=f"ctx_ps{v}")
                          for v in range(KC)]
                for mi in range(MI):
                    sim_ps = psS.tile([P, NT], f32, tag="sim", name="sim_ps")
                    for kc in range(KC):
                        nc.tensor.matmul(
                            sim_ps[:],
                            lhsT=k_sb[kc][:, mi * P:(mi + 1) * P],
                            rhs=k_sb[kc][:, nj * NT:(nj + 1) * NT],
                            start=(kc == 0), stop=(kc == KC - 1))
                    e_t = epool.tile([P, NT], bf16, tag="e", name="e_t")
                    col = mi * NJ + nj
                    nc.scalar.activation(
                        e_t[:], sim_ps[:], AF.Exp,
                        accum_out=rsparts_sb[:, col:col + 1])
                    for v in range(KC):
                        nc.tensor.matmul(
                            ctx_ps[v][:],
                            lhsT=vT_sb[:, mi * V + v * P: mi * V + (v + 1) * P],
                            rhs=e_t[:],
                            start=(mi == 0), stop=(mi == MI - 1))
                for v in range(KC):
                    nc.vector.tensor_copy(
                        ctxr_sb[v][:, nj * NT:(nj + 1) * NT], ctx_ps[v][:])

        # ---- finalize: rowsums -> recip -> broadcast -> normalize -> out proj
        with tc.tile_pool(name="psF", bufs=2, space="PSUM") as psF, \
                tc.tile_pool(name="psT", bufs=1, space="PSUM") as psT, \
                tc.tile_pool(name="psO", bufs=3, space="PSUM") as psO, \
                tc.tile_pool(name="fin", bufs=3) as fin:
            for mi in range(MI):
                nc.vector.tensor_reduce(
                    rs_sb[:, mi:mi + 1],
                    rsparts_sb[:, mi * NJ:(mi + 1) * NJ],
                    axis=mybir.AxisListType.X, op=mybir.AluOpType.add)
            tp_ps = psT.tile([MI, P], f32, tag="tp", name="tp_ps")
            nc.tensor.transpose(tp_ps[:], rs_sb[:], ident[:])
            nc.vector.reciprocal(recipT_sb[:], tp_ps[:])
            # flatten [32, 128] -> [1, 4096] row vector (SBUF->SBUF DMA)
            nc.sync.dma_start(out=reciprow_sb[:], in_=recipT_sb[:])

            for nj in range(NJ):
                bc_ps = psF.tile([P, NT], f32, tag="bc", name="bc_ps")
                nc.tensor.matmul(
                    bc_ps[:], lhsT=ones32[:],
                    rhs=reciprow_sb[:, nj * NT:(nj + 1) * NT],
                    start=True, stop=True)
                cn = [fin.tile([P, NT], bf16, tag=f"cn{v}", name=f"cn{v}")
                      for v in range(KC)]
                for v in range(KC):
                    nc.vector.tensor_mul(
                        cn[v][:], ctxr_sb[v][:, nj * NT:(nj + 1) * NT],
                        bc_ps[:])
                for oc in range(O // P):
                    op_ps = psO.tile([P, NT], f32, tag="op", name="op_ps")
                    for v in range(KC):
                        nc.tensor.matmul(
                            op_ps[:],
                            lhsT=wW_sb[v][:, oc * P:(oc + 1) * P],
                            rhs=cn[v][:],
                            start=(v == 0), stop=(v == KC - 1))
                    ot = fin.tile([P, NT], f32, tag="ot", name="ot")
                    nc.scalar.activation(ot[:], op_ps[:], AF.Identity,
                                         bias=bW_sb[:, oc:oc + 1])
                    nc.sync.dma_start(
                        out=out_d[oc * P:(oc + 1) * P, nj * NT:(nj + 1) * NT],
                        in_=ot[:])
    nc.compile()
    return nc


def _get_compiled():
    global _COMPILED
    if _COMPILED is None:
        _COMPILED = _build()
    return _COMPILED


def _make_in_maps(x, wv, bv, wk, bk, gamma, beta, rmean, rvar, wW, bW):
    x = np.asarray(x, dtype=np.float32)
    s = np.asarray(gamma, np.float32) / np.sqrt(np.asarray(rvar, np.float32) + BN_EPS)
    kscale = (s / 4.0).astype(np.float32).reshape(K, 1)
    kbias = (((np.asarray(bk, np.float32) - np.asarray(rmean, np.float32)) * s
              + np.asarray(beta, np.float32)) / 4.0).astype(np.float32).reshape(K, 1)
    shared = {
        "wkT16": np.ascontiguousarray(np.asarray(wk, np.float32).T).astype(BF16),
        "wvT16": np.ascontiguousarray(np.asarray(wv, np.float32).T).astype(BF16),
        "wWT16": np.ascontiguousarray(np.asarray(wW, np.float32).T).astype(BF16),
        "kscale": kscale,
        "kbias": kbias,
        "bvrow": np.asarray(bv, np.float32).reshape(1, V),
        "bW32": np.asarray(bW, np.float32).reshape(O, 1),
    }
    in_maps = []
    for b in range(B):
        m = dict(shared)
        m["x16"] = np.ascontiguousarray(x[b].reshape(C, N)).astype(BF16)
        in_maps.append(m)
    return in_maps


def _run(inputs, trace=False):
    from concourse.bass_utils import run_bass_kernel_spmd
    nc = _get_compiled()
    in_maps = _make_in_maps(**inputs)
    res = run_bass_kernel_spmd(nc, in_maps, list(range(B)), trace=trace)
    outs = [np.asarray(res.results[b]["out"], dtype=np.float32).reshape(O, H, W)
            for b in range(B)]
    return np.stack(outs), res


def kernel(x, wv, bv, wk, bk, gamma, beta, rmean, rvar, wW, bW):
    out, _ = _run(dict(x=x, wv=wv, bv=bv, wk=wk, bk=bk, gamma=gamma, beta=beta,
                       rmean=rmean, rvar=rvar, wW=wW, bW=bW))
    return out


# revision 9
# speedup vs baseline: 1.2046x; 1.2046x over previous
"""Trainium2 Bass kernel for BaseAttentionBlock (B=8, C=512, HxW=64x64, K=V=256, O=512).

Strategy: data-parallel over batch B across the 8 NeuronCores (one batch element
per core, SPMD, no collectives). Per core:

  k' = relu(s*(wk@x) + b2)/4 (BN folded on host, 1/sqrt(K) folded as 1/4 into k')
  vT = (x^T @ wv^T) + bv     computed directly in [m, v] layout (no transposes)
  E  = exp(k'^T k')          symmetric, so the [128m x 512n] tile computed in the
                             fused loop is simultaneously the [m, n]-layout rhs the
                             ctx matmul needs -> single pass, no transposes, no
                             DRAM round trip, rowsums fused into exp via accum_out
  ctx_raw[v, n] = sum_m vT[m, v] * E[m, n]   accumulated in PSUM per 512-wide n chunk
  out_raw = wW @ ctx_raw     (out-proj inside the main loop, unnormalized)
  out = out_raw * (1/rowsum) + bW   (softmax normalization deferred to the end)

k', E, vT are stored fp8e4m3 and the sim/ctx matmuls use DoubleRow perf mode
(contraction 256 in one pass, 2x PE throughput); fp32 accumulation in PSUM.
Projections and out-proj in bf16. Measured rel err ~8e-3 (bf16 variant: 1.5e-3).
"""

import numpy as np
import ml_dtypes

BN_EPS = 1e-5
B, C, H, W = 8, 512, 64, 64
N = H * W  # 4096
K = 256
V = 256
O = 512
P = 128  # partitions
NT = 512  # free-dim tile
CC = C // P  # 4 contraction chunks for projections
KC = K // P  # 2 kch chunks
MI = N // P  # 32 m-chunks of 128
NQ = MI // 2  # 16 m-chunk pairs (DoubleRow contraction = 256)
NJ = N // NT  # 8 n-chunks of 512
OC = O // P  # 4 output chunks
BF16 = ml_dtypes.bfloat16

_COMPILED = None


def _build():
    import concourse.bass as bass
    import concourse.tile as tile
    import concourse.mybir as mybir
    from concourse import bacc, masks
    from contextlib import ExitStack

    f32 = mybir.dt.float32
    bf16 = mybir.dt.bfloat16
    f8 = mybir.dt.float8e4
    AF = mybir.ActivationFunctionType
    DR = mybir.MatmulPerfMode.DoubleRow

    nc = bacc.Bacc(trn_type="TRN2", target_bir_lowering=False, debug=False,
                   num_devices=B)

    x_d = nc.dram_tensor("x16", [C, N], bf16, kind="ExternalInput").ap()
    wkT_d = nc.dram_tensor("wkT16", [C, K], bf16, kind="ExternalInput").ap()
    wvT_d = nc.dram_tensor("wvT16", [C, V], bf16, kind="ExternalInput").ap()
    wWT_d = nc.dram_tensor("wWT16", [V, O], bf16, kind="ExternalInput").ap()
    ks_d = nc.dram_tensor("kscale", [K, 1], f32, kind="ExternalInput").ap()
    kb_d = nc.dram_tensor("kbias", [K, 1], f32, kind="ExternalInput").ap()
    bv_d = nc.dram_tensor("bvrow", [1, V], f32, kind="ExternalInput").ap()
    bW_d = nc.dram_tensor("bW32", [O, 1], f32, kind="ExternalInput").ap()
    out_d = nc.dram_tensor("out", [O, N], f32, kind="ExternalOutput").ap()

    with tile.TileContext(nc) as tc, ExitStack() as ctx:
        const = ctx.enter_context(tc.tile_pool(name="const", bufs=1))

        # ---- persistent SBUF tensors ----
        x_sb = [const.tile([P, N], bf16, tag=f"x{c}", name=f"x_sb{c}")
                for c in range(CC)]
        wk_sb = [const.tile([P, K], bf16, tag=f"wk{c}", name=f"wk_sb{c}")
                 for c in range(CC)]
        wv_sb = [const.tile([P, V], bf16, tag=f"wv{c}", name=f"wv_sb{c}")
                 for c in range(CC)]
        wW_sb = [const.tile([P, O], bf16, tag=f"wW{v}", name=f"wW_sb{v}")
                 for v in range(KC)]
        ks_sb = const.tile([P, KC], f32, tag="ks", name="ks_sb")
        kb_sb = const.tile([P, KC], f32, tag="kb", name="kb_sb")
        bvrow_sb = const.tile([1, V], f32, tag="bvrow", name="bvrow_sb")
        bW_sb = const.tile([P, OC], f32, tag="bW", name="bW_sb")
        ones32 = const.tile([1, P], f32, tag="ones32", name="ones32")
        ident = const.tile([P, P], f32, tag="ident", name="ident")

        # k' fp8, both kch chunks in one tile (chunk kc at free offset kc*N)
        # -> DoubleRow lhsT/rhs views [P, 2, *]
        k2_sb = const.tile([P, KC * N], f8, tag="k2", name="k2_sb")
        # vT fp8 [m, v]: chunk mi occupies cols [mi*V, (mi+1)*V); an mi pair
        # q is the contiguous [P, 2, V] block at q*2*V
        vT_sb = const.tile([P, MI * V], f8, tag="vT", name="vT_sb")
        outr_sb = [const.tile([P, N], f32, tag=f"outr{oc}", name=f"outr_sb{oc}")
                   for oc in range(OC)]
        rsparts_sb = const.tile([P, MI * NJ], f32, tag="rsparts",
                                name="rsparts_sb")
        rs_sb = const.tile([P, MI], f32, tag="rs", name="rs_sb")
        recipT_sb = const.tile([MI, P], f32, tag="recipT", name="recipT_sb")
        reciprow_sb = const.tile([1, N], f32, tag="reciprow", name="reciprow_sb")
        bvbc_sb = const.tile([P, V], f32, tag="bvbc", name="bvbc_sb")

        # ---- input DMAs (x split by column quarters so compute can start
        # while the tail is still in flight) ----
        XQ = 4
        xq = N // XQ
        for q in range(XQ):
            for c in range(CC):
                nc.sync.dma_start(
                    out=x_sb[c][:, q * xq:(q + 1) * xq],
                    in_=x_d[c * P:(c + 1) * P, q * xq:(q + 1) * xq])
        for c in range(CC):
            nc.sync.dma_start(out=wk_sb[c][:], in_=wkT_d[c * P:(c + 1) * P, :])
            nc.sync.dma_start(out=wv_sb[c][:], in_=wvT_d[c * P:(c + 1) * P, :])
        for v in range(KC):
            nc.sync.dma_start(out=wW_sb[v][:], in_=wWT_d[v * P:(v + 1) * P, :])
        for kc in range(KC):
            nc.sync.dma_start(out=ks_sb[:, kc:kc + 1],
                              in_=ks_d[kc * P:(kc + 1) * P, :])
            nc.sync.dma_start(out=kb_sb[:, kc:kc + 1],
                              in_=kb_d[kc * P:(kc + 1) * P, :])
        nc.sync.dma_start(out=bvrow_sb[:], in_=bv_d[:])
        for oc in range(OC):
            nc.sync.dma_start(out=bW_sb[:, oc:oc + 1],
                              in_=bW_d[oc * P:(oc + 1) * P, :])
        nc.gpsimd.memset(ones32[:], 1.0)
        masks.make_identity(nc, ident[:])
        # bv broadcast [P, V] via stride-0 DMA read from DRAM
        nc.sync.dma_start(out=bvbc_sb[:], in_=bv_d[:].to_broadcast((P, V)))

        # ---- phase A: projections ----
        with tc.tile_pool(name="psA", bufs=4, space="PSUM") as psA:
            # k' = relu(ks * (wk@x) + kb) -> fp8, into k2 layout
            for kc in range(KC):
                for nj in range(NJ):
                    kp_ps = psA.tile([P, NT], f32, tag="ps", name="kp_ps")
                    for c in range(CC):
                        nc.tensor.matmul(
                            kp_ps[:],
                            lhsT=wk_sb[c][:, kc * P:(kc + 1) * P],
                            rhs=x_sb[c][:, nj * NT:(nj + 1) * NT],
                            start=(c == 0), stop=(c == CC - 1))
                    nc.scalar.activation(
                        k2_sb[:, kc * N + nj * NT: kc * N + (nj + 1) * NT],
                        kp_ps[:], AF.Relu, bias=kb_sb[:, kc:kc + 1],
                        scale=ks_sb[:, kc:kc + 1])

            # vT = x^T @ wvT + bv -> fp8, [m, v] layout
            for mi in range(MI):
                vp_ps = psA.tile([P, NT], f32, tag="ps", name="vp_ps")
                for c in range(CC):
                    nc.tensor.matmul(
                        vp_ps[:, :V],
                        lhsT=x_sb[c][:, mi * P:(mi + 1) * P],
                        rhs=wv_sb[c][:],
                        start=(c == 0), stop=(c == CC - 1))
                nc.vector.tensor_add(vT_sb[:, mi * V:(mi + 1) * V],
                                     vp_ps[:, :V], bvbc_sb[:])

        # DoubleRow contraction views
        k2v = k2_sb[:].rearrange("p (ko n) -> p ko n", ko=KC)      # [P, 2, N]
        vTv = vT_sb[:].rearrange("p (q ko v) -> p q ko v", q=NQ, ko=2)

        # ---- main fused loop ----
        with tc.tile_pool(name="psS", bufs=3, space="PSUM") as psS, \
                tc.tile_pool(name="psC", bufs=3, space="PSUM") as psC, \
                tc.tile_pool(name="psO", bufs=2, space="PSUM") as psO, \
                tc.tile_pool(name="epool", bufs=4) as epool, \
                tc.tile_pool(name="cnpool", bufs=4) as cnpool:
            for nj in range(NJ):
                ctx_ps = [psC.tile([P, NT], f32, tag="ctx", name=f"ctx_ps{v}")
                          for v in range(KC)]
                e2 = None
                for mi in range(MI):
                    q, half = divmod(mi, 2)
                    # sim tile [m-chunk mi, n-chunk nj], contraction 256 in
                    # one DoubleRow pass
                    sim_ps = psS.tile([P, NT], f32, tag="sim", name="sim_ps")
                    nc.tensor.matmul(
                        sim_ps[:],
                        lhsT=k2v[:, :, mi * P:(mi + 1) * P],
                        rhs=k2v[:, :, nj * NT:(nj + 1) * NT],
                        start=True, stop=True, perf_mode=DR)
                    if half == 0:
                        e2 = epool.tile([P, 2 * NT], f8, tag="e", name="e2")
                    col = mi * NJ + nj
                    nc.scalar.activation(
                        e2[:, half * NT:(half + 1) * NT], sim_ps[:], AF.Exp,
                        accum_out=rsparts_sb[:, col:col + 1])
                    if half == 1:
                        e2v = e2.rearrange("p (ko n) -> p ko n", ko=2)
                        for v in range(KC):
                            nc.tensor.matmul(
                                ctx_ps[v][:],
                                lhsT=vTv[:, q, :, v * P:(v + 1) * P],
                                rhs=e2v[:],
                                start=(q == 0), stop=(q == NQ - 1),
                                perf_mode=DR)
                # out-proj for this n-chunk (unnormalized), inside the loop
                cn = [cnpool.tile([P, NT], bf16, tag=f"cn{v}", name=f"cn{v}")
                      for v in range(KC)]
                for v in range(KC):
                    nc.vector.tensor_copy(cn[v][:], ctx_ps[v][:])
                for oc in range(OC):
                    op_ps = psO.tile([P, NT], f32, tag="op", name="op_ps")
                    for v in range(KC):
                        nc.tensor.matmul(
                            op_ps[:],
                            lhsT=wW_sb[v][:, oc * P:(oc + 1) * P],
                            rhs=cn[v][:],
                            start=(v == 0), stop=(v == KC - 1))
                    nc.vector.tensor_copy(
                        outr_sb[oc][:, nj * NT:(nj + 1) * NT], op_ps[:])

        # ---- finalize: rowsums -> recip -> broadcast -> scale + bias -> out
        with tc.tile_pool(name="psF", bufs=2, space="PSUM") as psF, \
                tc.tile_pool(name="psT", bufs=1, space="PSUM") as psT, \
                tc.tile_pool(name="fin", bufs=6) as fin:
            for mi in range(MI):
                nc.vector.tensor_reduce(
                    rs_sb[:, mi:mi + 1],
                    rsparts_sb[:, mi * NJ:(mi + 1) * NJ],
                    axis=mybir.AxisListType.X, op=mybir.AluOpType.add)
            tp_ps = psT.tile([MI, P], f32, tag="tp", name="tp_ps")
            nc.tensor.transpose(tp_ps[:], rs_sb[:], ident[:])
            nc.vector.reciprocal(recipT_sb[:], tp_ps[:])
            nc.sync.dma_start(out=reciprow_sb[:], in_=recipT_sb[:])

            for nj in range(NJ):
                bc_ps = psF.tile([P, NT], f32, tag="bc", name="bc_ps")
                nc.tensor.matmul(
                    bc_ps[:], lhsT=ones32[:],
                    rhs=reciprow_sb[:, nj * NT:(nj + 1) * NT],
                    start=True, stop=True)
                for oc in range(OC):
                    ft = fin.tile([P, NT], f32, tag="ft", name="ft")
                    nc.vector.tensor_mul(
                        ft[:], outr_sb[oc][:, nj * NT:(nj + 1) * NT], bc_ps[:])
                    ot = fin.tile([P, NT], f32, tag="ot", name="ot")
                    nc.scalar.activation(ot[:], ft[:], AF.Identity,
                                         bias=bW_sb[:, oc:oc + 1])
                    nc.sync.dma_start(
                        out=out_d[oc * P:(oc + 1) * P, nj * NT:(nj + 1) * NT],
                        in_=ot[:])
    nc.compile()
    return nc


def _get_compiled():
    global _COMPILED
    if _COMPILED is None:
        _COMPILED = _build()
    return _COMPILED


def _make_in_maps(x, wv, bv, wk, bk, gamma, beta, rmean, rvar, wW, bW):
    x = np.asarray(x, dtype=np.float32)
    s = np.asarray(gamma, np.float32) / np.sqrt(np.asarray(rvar, np.float32) + BN_EPS)
    kscale = (s / 4.0).astype(np.float32).reshape(K, 1)
    kbias = (((np.asarray(bk, np.float32) - np.asarray(rmean, np.float32)) * s
              + np.asarray(beta, np.float32)) / 4.0).astype(np.float32).reshape(K, 1)
    shared = {
        "wkT16": np.ascontiguousarray(np.asarray(wk, np.float32).T).astype(BF16),
        "wvT16": np.ascontiguousarray(np.asarray(wv, np.float32).T).astype(BF16),
        "wWT16": np.ascontiguousarray(np.asarray(wW, np.float32).T).astype(BF16),
        "kscale": kscale,
        "kbias": kbias,
        "bvrow": np.asarray(bv, np.float32).reshape(1, V),
        "bW32": np.asarray(bW, np.float32).reshape(O, 1),
    }
    in_maps = []
    for b in range(B):
        m = dict(shared)
        m["x16"] = np.ascontiguousarray(x[b].reshape(C, N)).astype(BF16)
        in_maps.append(m)
    return in_maps


def _run(inputs, trace=False):
    from concourse.bass_utils import run_bass_kernel_spmd
    nc = _get_compiled()
    in_maps = _make_in_maps(**inputs)
    res = run_bass_kernel_spmd(nc, in_maps, list(range(B)), trace=trace)
    outs = [np.asarray(res.results[b]["out"], dtype=np.float32).reshape(O, H, W)
            for b in range(B)]
    return np.stack(outs), res


def kernel(x, wv, bv, wk, bk, gamma, beta, rmean, rvar, wW, bW):
    out, _ = _run(dict(x=x, wv=wv, bv=bv, wk=wk, bk=bk, gamma=gamma, beta=beta,
                       rmean=rmean, rvar=rvar, wW=wW, bW=bW))
    return out


# revision 10
# speedup vs baseline: 1.2370x; 1.0269x over previous
"""Trainium2 Bass kernel for BaseAttentionBlock (B=8, C=512, HxW=64x64, K=V=256, O=512).

Strategy: data-parallel over batch B across the 8 NeuronCores (one batch element
per core, SPMD, no collectives). Per core:

  k' = relu(s*(wk@x) + b2)/4 (BN folded on host, 1/sqrt(K) folded as 1/4 into k')
  vT = (x^T @ wv^T) + bv     computed directly in [m, v] layout (no transposes)
  E  = exp(k'^T k')          symmetric, so the [128m x 512n] tile computed in the
                             fused loop is simultaneously the [m, n]-layout rhs the
                             ctx matmul needs -> single pass, no transposes, no
                             DRAM round trip, rowsums fused into exp via accum_out
  ctx_raw[v, n] = sum_m vT[m, v] * E[m, n]   accumulated in PSUM per 512-wide n chunk
  out_raw = wW @ ctx_raw     (out-proj inside the main loop, unnormalized)
  out = out_raw * (1/rowsum) + bW   (softmax normalization deferred to the end)

k', E, vT are stored fp8e4m3 and the sim/ctx matmuls use DoubleRow perf mode
(contraction 256 in one pass, 2x PE throughput); fp32 accumulation in PSUM.
Projections and out-proj in bf16. Measured rel err ~8e-3 (bf16 variant: 1.5e-3).
"""

import numpy as np
import ml_dtypes

BN_EPS = 1e-5
B, C, H, W = 8, 512, 64, 64
N = H * W  # 4096
K = 256
V = 256
O = 512
P = 128  # partitions
NT = 512  # free-dim tile
CC = C // P  # 4 contraction chunks for projections
KC = K // P  # 2 kch chunks
MI = N // P  # 32 m-chunks of 128
NQ = MI // 2  # 16 m-chunk pairs (DoubleRow contraction = 256)
NJ = N // NT  # 8 n-chunks of 512
OC = O // P  # 4 output chunks
BF16 = ml_dtypes.bfloat16

_COMPILED = None


def _build():
    import concourse.bass as bass
    import concourse.tile as tile
    import concourse.mybir as mybir
    from concourse import bacc, masks
    from contextlib import ExitStack

    f32 = mybir.dt.float32
    bf16 = mybir.dt.bfloat16
    f8 = mybir.dt.float8e4
    AF = mybir.ActivationFunctionType
    DR = mybir.MatmulPerfMode.DoubleRow

    nc = bacc.Bacc(trn_type="TRN2", target_bir_lowering=False, debug=False,
                   num_devices=B)

    x_d = nc.dram_tensor("x16", [C, N], bf16, kind="ExternalInput").ap()
    wkT_d = nc.dram_tensor("wkT16", [C, K], bf16, kind="ExternalInput").ap()
    wvT_d = nc.dram_tensor("wvT16", [C, V], bf16, kind="ExternalInput").ap()
    wWT_d = nc.dram_tensor("wWT16", [V, O], bf16, kind="ExternalInput").ap()
    ks_d = nc.dram_tensor("kscale", [K, 1], f32, kind="ExternalInput").ap()
    kb_d = nc.dram_tensor("kbias", [K, 1], f32, kind="ExternalInput").ap()
    bv_d = nc.dram_tensor("bvrow", [1, V], f32, kind="ExternalInput").ap()
    bW_d = nc.dram_tensor("bW32", [O, 1], f32, kind="ExternalInput").ap()
    out_d = nc.dram_tensor("out", [O, N], f32, kind="ExternalOutput").ap()

    with tile.TileContext(nc) as tc, ExitStack() as ctx:
        const = ctx.enter_context(tc.tile_pool(name="const", bufs=1))

        # ---- persistent SBUF tensors ----
        x_sb = [const.tile([P, N], bf16, tag=f"x{c}", name=f"x_sb{c}")
                for c in range(CC)]
        wk_sb = [const.tile([P, K], bf16, tag=f"wk{c}", name=f"wk_sb{c}")
                 for c in range(CC)]
        wv_sb = [const.tile([P, V], bf16, tag=f"wv{c}", name=f"wv_sb{c}")
                 for c in range(CC)]
        wW_sb = [const.tile([P, O], bf16, tag=f"wW{v}", name=f"wW_sb{v}")
                 for v in range(KC)]
        ks_sb = const.tile([P, KC], f32, tag="ks", name="ks_sb")
        kb_sb = const.tile([P, KC], f32, tag="kb", name="kb_sb")
        bvrow_sb = const.tile([1, V], f32, tag="bvrow", name="bvrow_sb")
        bW_sb = const.tile([P, OC], f32, tag="bW", name="bW_sb")
        ones32 = const.tile([1, P], f32, tag="ones32", name="ones32")
        ident = const.tile([P, P], f32, tag="ident", name="ident")

        # k' fp8, both kch chunks in one tile (chunk kc at free offset kc*N)
        # -> DoubleRow lhsT/rhs views [P, 2, *]
        k2_sb = const.tile([P, KC * N], f8, tag="k2", name="k2_sb")
        # vT fp8 [m, v]: chunk mi occupies cols [mi*V, (mi+1)*V); an mi pair
        # q is the contiguous [P, 2, V] block at q*2*V
        vT_sb = const.tile([P, MI * V], f8, tag="vT", name="vT_sb")
        outr_sb = [const.tile([P, N], f32, tag=f"outr{oc}", name=f"outr_sb{oc}")
                   for oc in range(OC)]
        rsparts_sb = const.tile([P, MI * NJ], f32, tag="rsparts",
                                name="rsparts_sb")
        rs_sb = const.tile([P, MI], f32, tag="rs", name="rs_sb")
        recipT_sb = const.tile([MI, P], f32, tag="recipT", name="recipT_sb")
        reciprow_sb = const.tile([1, N], f32, tag="reciprow", name="reciprow_sb")
        bvbc_sb = const.tile([P, V], f32, tag="bvbc", name="bvbc_sb")

        # ---- input DMAs: small weights first, then x by column quarters so
        # the projection + nj=0 compute wave can ride behind the transfer ----
        for c in range(CC):
            nc.sync.dma_start(out=wk_sb[c][:], in_=wkT_d[c * P:(c + 1) * P, :])
            nc.sync.dma_start(out=wv_sb[c][:], in_=wvT_d[c * P:(c + 1) * P, :])
        for v in range(KC):
            nc.sync.dma_start(out=wW_sb[v][:], in_=wWT_d[v * P:(v + 1) * P, :])
        for kc in range(KC):
            nc.sync.dma_start(out=ks_sb[:, kc:kc + 1],
                              in_=ks_d[kc * P:(kc + 1) * P, :])
            nc.sync.dma_start(out=kb_sb[:, kc:kc + 1],
                              in_=kb_d[kc * P:(kc + 1) * P, :])
        nc.sync.dma_start(out=bvrow_sb[:], in_=bv_d[:])
        for oc in range(OC):
            nc.sync.dma_start(out=bW_sb[:, oc:oc + 1],
                              in_=bW_d[oc * P:(oc + 1) * P, :])
        nc.gpsimd.memset(ones32[:], 1.0)
        masks.make_identity(nc, ident[:])
        # bv broadcast [P, V] via stride-0 DMA read from DRAM
        nc.sync.dma_start(out=bvbc_sb[:], in_=bv_d[:].to_broadcast((P, V)))
        XQ = 4
        xq = N // XQ
        for q in range(XQ):
            for c in range(CC):
                nc.sync.dma_start(
                    out=x_sb[c][:, q * xq:(q + 1) * xq],
                    in_=x_d[c * P:(c + 1) * P, q * xq:(q + 1) * xq])

        # DoubleRow contraction views
        k2v = k2_sb[:].rearrange("p (ko n) -> p ko n", ko=KC)      # [P, 2, N]
        vTv = vT_sb[:].rearrange("p (q ko v) -> p q ko v", q=NQ, ko=2)

        # ---- fused program: projections + nj=0 woven into the x-DMA wave,
        # then the remaining n-chunks ----
        with tc.tile_pool(name="psP", bufs=2, space="PSUM") as psP, \
                tc.tile_pool(name="psS", bufs=4, space="PSUM") as psS, \
                tc.tile_pool(name="psC", bufs=2, space="PSUM") as psC, \
                tc.tile_pool(name="epool", bufs=6) as epool, \
                tc.tile_pool(name="cnpool", bufs=4) as cnpool:

            def kproj(kc, nj):
                kp_ps = psP.tile([P, NT], f32, tag="ps", name="kp_ps")
                for c in range(CC):
                    nc.tensor.matmul(
                        kp_ps[:],
                        lhsT=wk_sb[c][:, kc * P:(kc + 1) * P],
                        rhs=x_sb[c][:, nj * NT:(nj + 1) * NT],
                        start=(c == 0), stop=(c == CC - 1))
                nc.scalar.activation(
                    k2_sb[:, kc * N + nj * NT: kc * N + (nj + 1) * NT],
                    kp_ps[:], AF.Relu, bias=kb_sb[:, kc:kc + 1],
                    scale=ks_sb[:, kc:kc + 1])

            def vproj(mi):
                vp_ps = psP.tile([P, NT], f32, tag="ps", name="vp_ps")
                for c in range(CC):
                    nc.tensor.matmul(
                        vp_ps[:, :V],
                        lhsT=x_sb[c][:, mi * P:(mi + 1) * P],
                        rhs=wv_sb[c][:],
                        start=(c == 0), stop=(c == CC - 1))
                nc.vector.tensor_add(vT_sb[:, mi * V:(mi + 1) * V],
                                     vp_ps[:, :V], bvbc_sb[:])

            loop_state = {}

            def loop_iter(nj, mi):
                q, half = divmod(mi, 2)
                sim_ps = psS.tile([P, NT], f32, tag="sim", name="sim_ps")
                nc.tensor.matmul(
                    sim_ps[:],
                    lhsT=k2v[:, :, mi * P:(mi + 1) * P],
                    rhs=k2v[:, :, nj * NT:(nj + 1) * NT],
                    start=True, stop=True, perf_mode=DR)
                if half == 0:
                    loop_state["e2"] = epool.tile([P, 2 * NT], f8, tag="e",
                                                  name="e2")
                e2 = loop_state["e2"]
                col = mi * NJ + nj
                nc.scalar.activation(
                    e2[:, half * NT:(half + 1) * NT], sim_ps[:], AF.Exp,
                    accum_out=rsparts_sb[:, col:col + 1])
                if half == 1:
                    e2v = e2.rearrange("p (ko n) -> p ko n", ko=2)
                    for v in range(KC):
                        nc.tensor.matmul(
                            loop_state["ctx"][v][:],
                            lhsT=vTv[:, q, :, v * P:(v + 1) * P],
                            rhs=e2v[:],
                            start=(q == 0), stop=(q == NQ - 1),
                            perf_mode=DR)

            def start_nj(nj):
                loop_state["ctx"] = [
                    psC.tile([P, NT], f32, tag="ctx", name=f"ctx_ps{v}")
                    for v in range(KC)]

            def finish_nj(nj):
                # out-proj for this n-chunk (unnormalized), inside the loop
                ctx_ps = loop_state["ctx"]
                cn = [cnpool.tile([P, NT], bf16, tag=f"cn{v}", name=f"cn{v}")
                      for v in range(KC)]
                for v in range(KC):
                    nc.vector.tensor_copy(cn[v][:], ctx_ps[v][:])
                for oc in range(OC):
                    op_ps = psP.tile([P, NT], f32, tag="ps", name="op_ps")
                    for v in range(KC):
                        nc.tensor.matmul(
                            op_ps[:],
                            lhsT=wW_sb[v][:, oc * P:(oc + 1) * P],
                            rhs=cn[v][:],
                            start=(v == 0), stop=(v == KC - 1))
                    nc.vector.tensor_copy(
                        outr_sb[oc][:, nj * NT:(nj + 1) * NT], op_ps[:])

            # quarter-by-quarter wave: kproj + vproj + nj=0 segment
            start_nj(0)
            for q in range(XQ):
                for kc in range(KC):
                    for nj in (2 * q, 2 * q + 1):
                        kproj(kc, nj)
                for mi in range(8 * q, 8 * q + 8):
                    vproj(mi)
                    loop_iter(0, mi)
            finish_nj(0)
            # remaining n-chunks
            for nj in range(1, NJ):
                start_nj(nj)
                for mi in range(MI):
                    loop_iter(nj, mi)
                finish_nj(nj)

        # ---- finalize: rowsums -> recip -> broadcast -> scale + bias -> out
        with tc.tile_pool(name="psF", bufs=2, space="PSUM") as psF, \
                tc.tile_pool(name="psT", bufs=1, space="PSUM") as psT, \
                tc.tile_pool(name="fin", bufs=6) as fin:
            for mi in range(MI):
                nc.vector.tensor_reduce(
                    rs_sb[:, mi:mi + 1],
                    rsparts_sb[:, mi * NJ:(mi + 1) * NJ],
                    axis=mybir.AxisListType.X, op=mybir.AluOpType.add)
            tp_ps = psT.tile([MI, P], f32, tag="tp", name="tp_ps")
            nc.tensor.transpose(tp_ps[:], rs_sb[:], ident[:])
            nc.vector.reciprocal(recipT_sb[:], tp_ps[:])
            nc.sync.dma_start(out=reciprow_sb[:], in_=recipT_sb[:])

            for nj in range(NJ):
                bc_ps = psF.tile([P, NT], f32, tag="bc", name="bc_ps")
                nc.tensor.matmul(
                    bc_ps[:], lhsT=ones32[:],
                    rhs=reciprow_sb[:, nj * NT:(nj + 1) * NT],
                    start=True, stop=True)
                for oc in range(OC):
                    ft = fin.tile([P, NT], f32, tag="ft", name="ft")
                    nc.vector.tensor_mul(
                        ft[:], outr_sb[oc][:, nj * NT:(nj + 1) * NT], bc_ps[:])
                    ot = fin.tile([P, NT], f32, tag="ot", name="ot")
                    nc.scalar.activation(ot[:], ft[:], AF.Identity,
                                         bias=bW_sb[:, oc:oc + 1])
                    nc.sync.dma_start(
                        out=out_d[oc * P:(oc + 1) * P, nj * NT:(nj + 1) * NT],
                        in_=ot[:])
    nc.compile()
    return nc


def _get_compiled():
    global _COMPILED
    if _COMPILED is None:
        _COMPILED = _build()
    return _COMPILED


def _make_in_maps(x, wv, bv, wk, bk, gamma, beta, rmean, rvar, wW, bW):
    x = np.asarray(x, dtype=np.float32)
    s = np.asarray(gamma, np.float32) / np.sqrt(np.asarray(rvar, np.float32) + BN_EPS)
    kscale = (s / 4.0).astype(np.float32).reshape(K, 1)
    kbias = (((np.asarray(bk, np.float32) - np.asarray(rmean, np.float32)) * s
              + np.asarray(beta, np.float32)) / 4.0).astype(np.float32).reshape(K, 1)
    shared = {
        "wkT16": np.ascontiguousarray(np.asarray(wk, np.float32).T).astype(BF16),
        "wvT16": np.ascontiguousarray(np.asarray(wv, np.float32).T).astype(BF16),
        "wWT16": np.ascontiguousarray(np.asarray(wW, np.float32).T).astype(BF16),
        "kscale": kscale,
        "kbias": kbias,
        "bvrow": np.asarray(bv, np.float32).reshape(1, V),
        "bW32": np.asarray(bW, np.float32).reshape(O, 1),
    }
    in_maps = []
    for b in range(B):
        m = dict(shared)
        m["x16"] = np.ascontiguousarray(x[b].reshape(C, N)).astype(BF16)
        in_maps.append(m)
    return in_maps


def _run(inputs, trace=False):
    from concourse.bass_utils import run_bass_kernel_spmd
    nc = _get_compiled()
    in_maps = _make_in_maps(**inputs)
    res = run_bass_kernel_spmd(nc, in_maps, list(range(B)), trace=trace)
    outs = [np.asarray(res.results[b]["out"], dtype=np.float32).reshape(O, H, W)
            for b in range(B)]
    return np.stack(outs), res


def kernel(x, wv, bv, wk, bk, gamma, beta, rmean, rvar, wW, bW):
    out, _ = _run(dict(x=x, wv=wv, bv=bv, wk=wk, bk=bk, gamma=gamma, beta=beta,
                       rmean=rmean, rvar=rvar, wW=wW, bW=bW))
    return out
